# revision 23
# baseline (speedup 1.0000x reference)
"""MoE gate (top-2 routing + seq-aux loss) on 8 Trainium2 NeuronCores.

Strategy (data-parallel over tokens, per sharding hint):
  - Flatten [B=4, S=8192] -> 32768 tokens; each of the 8 cores handles 4096
    tokens (exactly half of one batch row, so per-core aux partial sums
    belong to a single batch).
  - Host pre-transposes each token shard to xT [H=1024, 4096] so the device
    DMAs contiguous rows with H on partitions (PE contraction dim), avoiding
    an fp32 on-chip transpose.  The tiny router weight is host-transposed to
    wT [1024, 8] and replicated.
  - Device per core: fp32 matmul (xT chunk stationary [128h,128t], wT chunk
    moving [128h,8e]) accumulating logits [128t, 8e] in PSUM over 8 H-chunks;
    ScalarE exp; VectorE sum/reciprocal/top-8 sort + index; outputs top-2
    weights (softmax probs) + int32 indices; accumulates per-core partial
    sums of softmax probs and top-2 one-hot counts for the aux loss.
  - Host gathers, reassembles [B, S, 2] outputs, and finishes the scalar
    aux loss from the per-core partial sums.
"""

import os
import sys

import numpy as np

for _p in ("/opt/trn_rl_repo", "/root/.axon_site/_ro/trn_rl_repo"):
    if os.path.isdir(_p) and _p not in sys.path:
        sys.path.insert(0, _p)

import concourse.bacc as bacc
import concourse.mybir as mybir
from concourse.tile import TileContext
from concourse.bass_utils import run_bass_kernel_spmd

# Problem constants (hardcoded per spec).
B, S, H = 4, 8192, 1024
E, K = 8, 2
ALPHA = 0.1
N_CORES = 8
TOK_PER_CORE = (B * S) // N_CORES  # 4096
P = 128
NCH = H // P  # 8 contraction chunks

# Tunables
T_B = 1024                 # tokens per pipeline block
MB = T_B // P              # 128-token subtiles per block
NB = TOK_PER_CORE // T_B   # blocks per core
SB = 512                   # tokens per PSUM sub-block (one matmul stream)
NSB = T_B // SB

F32 = mybir.dt.float32
F32R = mybir.dt.float32r
I32 = mybir.dt.int32
U32 = mybir.dt.uint32

_COMPILED = {}


def _build():
    nc = bacc.Bacc("TRN2", target_bir_lowering=False)

    # hi/lo 11-bit (f32r) decomposition of the token shard and router weight:
    # logits = xh@wh + xh@wl + xl@wh  (xl@wl ~ 2^-24, dropped).  f32r streams
    # at full PE rate (1 cyc/row) vs fp32's 4, and products of 11-bit
    # mantissas are exact, so this is fp32-grade accuracy at 2x speed.
    xh_d = nc.dram_tensor("xh", [H, TOK_PER_CORE], F32R, kind="ExternalInput")
    xl_d = nc.dram_tensor("xl", [H, TOK_PER_CORE], F32R, kind="ExternalInput")
    whl_d = nc.dram_tensor("whl", [H, 2 * E], F32R, kind="ExternalInput")
    id16_d = nc.dram_tensor("id16", [2 * E, 2 * E], F32, kind="ExternalInput")
    wout_d = nc.dram_tensor("wout", [P, NB * MB * 2], F32, kind="ExternalOutput")
    iout_d = nc.dram_tensor("iout", [P, NB * MB * 2], I32, kind="ExternalOutput")
    sacc_d = nc.dram_tensor("sacc", [P, E], F32, kind="ExternalOutput")
    facc_d = nc.dram_tensor("facc", [P, E], F32, kind="ExternalOutput")

    with TileContext(nc) as tc:
        with (
            tc.tile_pool(name="xin", bufs=2) as xin_pool,
            tc.tile_pool(name="work", bufs=3) as work,
            tc.tile_pool(name="outs", bufs=3) as outs,
            tc.tile_pool(name="singles", bufs=1) as singles,
            tc.tile_pool(name="psum", bufs=2, space="PSUM") as psum_pool,
        ):
            # Router weight [w_hi | w_lo], H-chunk-major.
            wt_sb = singles.tile([P, NCH, 2 * E], F32R)
            nc.sync.dma_start(
                out=wt_sb, in_=whl_d.rearrange("(c p) e -> p c e", p=P)
            )
            id16_sb = singles.tile([2 * E, 2 * E], F32)
            nc.sync.dma_start(out=id16_sb, in_=id16_d[:, :])

            # Aux-loss accumulators.
            stot = singles.tile([P, MB, E], F32)
            ftot = singles.tile([P, MB, E], F32)
            nc.vector.memset(stot, 0.0)
            nc.vector.memset(ftot, 0.0)

            for j in range(NB):
                # One DMA per (H-chunk, hi/lo) so the first matmul can start
                # after ~1/16 of the block has landed.
                xh_sb = xin_pool.tile([P, NCH, T_B], F32R)
                xl_sb = xin_pool.tile([P, NCH, T_B], F32R)
                for c in range(NCH):
                    nc.sync.dma_start(
                        out=xh_sb[:, c, :],
                        in_=xh_d[c * P:(c + 1) * P, j * T_B:(j + 1) * T_B],
                    )
                    nc.sync.dma_start(
                        out=xl_sb[:, c, :],
                        in_=xl_d[c * P:(c + 1) * P, j * T_B:(j + 1) * T_B],
                    )

                # Per sub-block: rows 0:8 accumulate xh@wh then xl@wh; rows
                # 8:16 accumulate xh@wl (same stream as xh@wh via the packed
                # 16-column stationary).  The [16, 128] slices are PE-
                # transposed back to token-major and the hi+lo halves are
                # combined along the free dim afterwards.
                lg16_ps = psum_pool.tile([P, MB, 2 * E], F32)
                for s in range(NSB):
                    lt_ps = psum_pool.tile([2 * E, SB], F32, tag="ltps")
                    for c in range(NCH):
                        nc.tensor.matmul(
                            lt_ps,
                            lhsT=wt_sb[:, c, :],
                            rhs=xh_sb[:, c, s * SB:(s + 1) * SB],
                            start=(c == 0),
                            stop=False,
                        )
                    for c in range(NCH):
                        nc.tensor.matmul(
                            lt_ps[0:E, :],
                            lhsT=wt_sb[:, c, 0:E],
                            rhs=xl_sb[:, c, s * SB:(s + 1) * SB],
                            start=False,
                            stop=(c == NCH - 1),
                            skip_group_check=True,
                        )
                    lt_sb = work.tile([2 * E, SB], F32, tag="ltsb")
                    nc.scalar.copy(lt_sb, lt_ps)
                    for m4 in range(SB // P):
                        nc.tensor.transpose(
                            lg16_ps[:, s * (SB // P) + m4, :],
                            lt_sb[:, m4 * P:(m4 + 1) * P],
                            id16_sb,
                        )

                # logits = hi + lo halves, token-major, in SBUF.
                lg16 = work.tile([P, MB, 2 * E], F32)
                nc.scalar.copy(lg16, lg16_ps)
                logits_ps = work.tile([P, MB, E], F32, tag="lg")
                nc.vector.tensor_add(
                    logits_ps, lg16[:, :, 0:E], lg16[:, :, E:2 * E]
                )

                # softmax with per-token max subtraction (matches reference:
                # logits - max, then exp).  negate=True gives -max directly,
                # used as the per-partition ACT bias (one op per 128-token
                # subtile since bias is a per-partition scalar).
                nmx = work.tile([P, MB], F32)
                nc.vector.reduce_max(
                    nmx, logits_ps, axis=mybir.AxisListType.X, negate=True
                )
                ex = work.tile([P, MB, E], F32)
                for m in range(MB):
                    nc.scalar.activation(
                        ex[:, m, :],
                        logits_ps[:, m, :],
                        mybir.ActivationFunctionType.Exp,
                        bias=nmx[:, m:m + 1],
                    )

                z = work.tile([P, MB], F32)
                nc.vector.reduce_sum(z, ex, axis=mybir.AxisListType.X)
                r = work.tile([P, MB], F32)
                nc.vector.reciprocal(r, z)

                # full softmax scores (also feeds the aux-loss accumulator);
                # top-k runs on these so ties after rounding break like jax's.
                sc = work.tile([P, MB, E], F32)
                nc.vector.tensor_mul(sc, ex, r.to_broadcast([P, MB, E]))

                m8 = work.tile([P, MB, E], F32)
                i8 = work.tile([P, MB, E], U32)
                for m in range(MB):
                    nc.vector.max(m8[:, m, :], sc[:, m, :])
                for m in range(MB):
                    nc.vector.max_index(i8[:, m, :], m8[:, m, :], sc[:, m, :])

                wv = outs.tile([P, MB, 2], F32)
                nc.vector.tensor_copy(wv, m8[:, :, 0:2])
                iv = outs.tile([P, MB, 2], I32)
                nc.vector.tensor_copy(iv, i8[:, :, 0:2])

                nc.sync.dma_start(
                    out=wout_d.rearrange("p (j q) -> p j q", j=NB)[:, j, :],
                    in_=wv,
                )
                nc.sync.dma_start(
                    out=iout_d.rearrange("p (j q) -> p j q", j=NB)[:, j, :],
                    in_=iv,
                )

                # Aux partial sums: softmax probs and top-2 one-hot counts.
                nc.vector.tensor_add(stot, stot, sc)
                ft = work.tile([P, MB, E], F32)
                nc.vector.tensor_tensor(
                    out=ft,
                    in0=sc,
                    in1=m8[:, :, 1:2].to_broadcast([P, MB, E]),
                    op=mybir.AluOpType.is_ge,
                )
                nc.vector.tensor_add(ftot, ftot, ft)

            # Reduce the per-(m) accumulators over m -> [P, E].
            sacc_sb = singles.tile([P, E], F32)
            facc_sb = singles.tile([P, E], F32)
            nc.vector.reduce_sum(
                sacc_sb, stot.rearrange("p m e -> p e m"), axis=mybir.AxisListType.X
            )
            nc.vector.reduce_sum(
                facc_sb, ftot.rearrange("p m e -> p e m"), axis=mybir.AxisListType.X
            )
            nc.sync.dma_start(out=sacc_d[:, :], in_=sacc_sb)
            nc.sync.dma_start(out=facc_d[:, :], in_=facc_sb)

    nc.finalize()
    return nc


def get_nc():
    if "nc" not in _COMPILED:
        _COMPILED["nc"] = _build()
    return _COMPILED["nc"]


def _rne11(a: np.ndarray) -> np.ndarray:
    """Round fp32 to 11 explicit mantissa bits (f32r's input rounding, RNE)."""
    u = np.ascontiguousarray(a, dtype=np.float32).view(np.uint32)
    r = (u + np.uint32(0x7FF) + ((u >> np.uint32(12)) & np.uint32(1))) & np.uint32(
        0xFFFFF000
    )
    return r.view(np.float32)


def _host_prep(hidden_states: np.ndarray, weight: np.ndarray):
    x = np.ascontiguousarray(hidden_states, dtype=np.float32).reshape(B * S, H)
    w = weight.astype(np.float32, copy=False)
    wh = _rne11(w)
    wl = w - wh
    whl = np.ascontiguousarray(np.concatenate([wh.T, wl.T], axis=1))  # [H, 16]
    id16 = np.eye(2 * E, dtype=np.float32)
    xt = np.ascontiguousarray(x.T)  # [H, B*S]
    xh = _rne11(xt)
    xl = xt - xh
    in_maps = []
    for c in range(N_CORES):
        sl = slice(c * TOK_PER_CORE, (c + 1) * TOK_PER_CORE)
        in_maps.append(
            {
                "xh": np.ascontiguousarray(xh[:, sl]),
                "xl": np.ascontiguousarray(xl[:, sl]),
                "whl": whl,
                "id16": id16,
            }
        )
    return in_maps


def _host_finish(results):
    topk_w = np.empty((B * S, K), np.float32)
    topk_i = np.empty((B * S, K), np.int32)
    p_sum = np.zeros((B, E), np.float64)
    f_cnt = np.zeros((B, E), np.float64)
    for c, res in enumerate(results):
        # wout/iout layout: [p, j, m, q] for token j*T_B + m*128 + p
        w = res["wout"].reshape(P, NB, MB, K).transpose(1, 2, 0, 3).reshape(-1, K)
        i = res["iout"].reshape(P, NB, MB, K).transpose(1, 2, 0, 3).reshape(-1, K)
        topk_w[c * TOK_PER_CORE:(c + 1) * TOK_PER_CORE] = w
        topk_i[c * TOK_PER_CORE:(c + 1) * TOK_PER_CORE] = i
        b = c // (N_CORES // B)
        p_sum[b] += res["sacc"].astype(np.float64).sum(axis=0)
        f_cnt[b] += res["facc"].astype(np.float64).sum(axis=0)

    p = p_sum / S                      # scores.mean over seq  [B, E]
    f = (E / (K * S)) * f_cnt / S      # scaled one-hot counts, mean over seq
    aux = np.float32(ALPHA * np.mean((f * p).sum(axis=1)))

    return (
        topk_i.reshape(B, S, K),
        topk_w.reshape(B, S, K),
        aux,
    )


def kernel(hidden_states: np.ndarray, weight: np.ndarray, _trace=False,
           _tmpdir=None):
    nc = get_nc()
    in_maps = _host_prep(hidden_states, weight)
    res = run_bass_kernel_spmd(
        nc, in_maps, core_ids=list(range(N_CORES)), trace=_trace,
        tmpdir=_tmpdir,
    )
    out = _host_finish(res.results)
    if _trace:
        return out, res
    return out


# revision 36
# speedup vs baseline: 1.7645x; 1.7645x over previous
"""MoE gate (top-2 routing + seq-aux loss) on 8 Trainium2 NeuronCores.

Strategy (data-parallel over tokens, per sharding hint):
  - Flatten [B=4, S=8192] -> 32768 tokens; each of the 8 cores handles 4096
    tokens (exactly half of one batch row, so per-core aux partial sums
    belong to a single batch).
  - Host pre-transposes each token shard to xT [H=1024, 4096] so the device
    DMAs contiguous rows with H on partitions (PE contraction dim), avoiding
    an fp32 on-chip transpose.  The tiny router weight is host-transposed to
    wT [1024, 8] and replicated.
  - Device per core: fp32 matmul (xT chunk stationary [128h,128t], wT chunk
    moving [128h,8e]) accumulating logits [128t, 8e] in PSUM over 8 H-chunks;
    ScalarE exp; VectorE sum/reciprocal/top-8 sort + index; outputs top-2
    weights (softmax probs) + int32 indices; accumulates per-core partial
    sums of softmax probs and top-2 one-hot counts for the aux loss.
  - Host gathers, reassembles [B, S, 2] outputs, and finishes the scalar
    aux loss from the per-core partial sums.
"""

import os
import sys

import numpy as np

for _p in ("/opt/trn_rl_repo", "/root/.axon_site/_ro/trn_rl_repo"):
    if os.path.isdir(_p) and _p not in sys.path:
        sys.path.insert(0, _p)

import concourse.bacc as bacc
import concourse.mybir as mybir
from concourse.tile import TileContext
from concourse.bass_utils import run_bass_kernel_spmd

# Problem constants (hardcoded per spec).
B, S, H = 4, 8192, 1024
E, K = 8, 2
ALPHA = 0.1
N_CORES = 8
TOK_PER_CORE = (B * S) // N_CORES  # 4096
P = 128
NCH = H // P  # 8 contraction chunks

# Tunables
T_B = 1024                 # tokens per pipeline block
MB = T_B // P              # 128-token subtiles per block
NB = TOK_PER_CORE // T_B   # blocks per core
SB = 512                   # tokens per PSUM sub-block (one matmul stream)
NSB = T_B // SB

F32 = mybir.dt.float32
BF16 = mybir.dt.bfloat16
I32 = mybir.dt.int32
U32 = mybir.dt.uint32

_COMPILED = {}


def _build():
    nc = bacc.Bacc("TRN2", target_bir_lowering=False)

    # Exact fp32 path: weight-stationary matmul streams the token dim
    # (fp32 = 2 internal half-rate passes, but 512-row streams keep the PE
    # warm at 2.4 GHz and hide the tiny weight loads).
    xt_d = nc.dram_tensor("xt", [H, TOK_PER_CORE], F32, kind="ExternalInput")
    wt_d = nc.dram_tensor("wt", [H, E], F32, kind="ExternalInput")
    id8_d = nc.dram_tensor("id8", [E, E], F32, kind="ExternalInput")
    wout_d = nc.dram_tensor("wout", [P, NB * MB * 2], F32, kind="ExternalOutput")
    iout_d = nc.dram_tensor("iout", [P, NB * MB * 2], I32, kind="ExternalOutput")
    sacc_d = nc.dram_tensor("sacc", [P, E], F32, kind="ExternalOutput")
    facc_d = nc.dram_tensor("facc", [P, E], F32, kind="ExternalOutput")

    with TileContext(nc) as tc:
        with (
            tc.tile_pool(name="xin", bufs=3) as xin_pool,
            tc.tile_pool(name="work", bufs=3) as work,
            tc.tile_pool(name="outs", bufs=3) as outs,
            tc.tile_pool(name="singles", bufs=1) as singles,
            tc.tile_pool(name="psum", bufs=2, space="PSUM") as psum_pool,
        ):
            # Router weight, H-chunk-major: wt_sb[p, c, e] = wT[c*128+p, e]
            wt_sb = singles.tile([P, NCH, E], F32)
            nc.sync.dma_start(
                out=wt_sb, in_=wt_d.rearrange("(c p) e -> p c e", p=P)
            )
            id8_sb = singles.tile([E, E], F32)
            nc.sync.dma_start(out=id8_sb, in_=id8_d[:, :])

            # Aux-loss accumulators.
            stot = singles.tile([P, MB, E], F32)
            ftot = singles.tile([P, MB, E], F32)
            nc.vector.memset(stot, 0.0)
            nc.vector.memset(ftot, 0.0)

            for j in range(NB):
                # One DMA per H-chunk so the first matmul can start after
                # ~1/8 of the block has landed.
                xt_sb = xin_pool.tile([P, NCH, T_B], F32)
                for c in range(NCH):
                    nc.sync.dma_start(
                        out=xt_sb[:, c, :],
                        in_=xt_d[c * P:(c + 1) * P, j * T_B:(j + 1) * T_B],
                    )

                # logitsT [E, SB] per sub-block; tiny PE transposes flip
                # each 128-token slice back to token-major PSUM.
                logits_ps = psum_pool.tile([P, MB, E], F32)
                for s in range(NSB):
                    lt_ps = psum_pool.tile([E, SB], F32, tag="ltps")
                    for c in range(NCH):
                        nc.tensor.matmul(
                            lt_ps,
                            lhsT=wt_sb[:, c, :],
                            rhs=xt_sb[:, c, s * SB:(s + 1) * SB],
                            start=(c == 0),
                            stop=(c == NCH - 1),
                        )
                    lt_sb = work.tile([E, SB], F32, tag="ltsb")
                    nc.scalar.copy(lt_sb, lt_ps)
                    for m4 in range(SB // P):
                        nc.tensor.transpose(
                            logits_ps[:, s * (SB // P) + m4, :],
                            lt_sb[:, m4 * P:(m4 + 1) * P],
                            id8_sb,
                        )

                # softmax with per-token max subtraction (matches reference:
                # logits - max, then exp).  negate=True gives -max directly,
                # used as the per-partition ACT bias (one op per 128-token
                # subtile since bias is a per-partition scalar).
                nmx = work.tile([P, MB], F32)
                nc.vector.reduce_max(
                    nmx, logits_ps, axis=mybir.AxisListType.X, negate=True
                )
                ex = work.tile([P, MB, E], F32)
                for m in range(MB):
                    nc.scalar.activation(
                        ex[:, m, :],
                        logits_ps[:, m, :],
                        mybir.ActivationFunctionType.Exp,
                        bias=nmx[:, m:m + 1],
                    )

                z = work.tile([P, MB], F32)
                nc.vector.reduce_sum(z, ex, axis=mybir.AxisListType.X)
                r = work.tile([P, MB], F32)
                nc.vector.reciprocal(r, z)

                # full softmax scores (also feeds the aux-loss accumulator);
                # top-k runs on these so ties after rounding break like jax's.
                sc = work.tile([P, MB, E], F32)
                nc.vector.tensor_mul(sc, ex, r.to_broadcast([P, MB, E]))

                m8 = work.tile([P, MB, E], F32)
                i8 = work.tile([P, MB, E], U32)
                for m in range(MB):
                    nc.vector.max(m8[:, m, :], sc[:, m, :])
                for m in range(MB):
                    nc.vector.max_index(i8[:, m, :], m8[:, m, :], sc[:, m, :])

                wv = outs.tile([P, MB, 2], F32)
                nc.vector.tensor_copy(wv, m8[:, :, 0:2])
                iv = outs.tile([P, MB, 2], I32)
                nc.vector.tensor_copy(iv, i8[:, :, 0:2])

                # Outputs leave on the scalar engine's HWDGE ring so they
                # never block the sync ring's input prefetch (a waiting DMA
                # at the ring head stalls everything queued behind it).
                nc.scalar.dma_start(
                    out=wout_d.rearrange("p (j q) -> p j q", j=NB)[:, j, :],
                    in_=wv,
                )
                nc.scalar.dma_start(
                    out=iout_d.rearrange("p (j q) -> p j q", j=NB)[:, j, :],
                    in_=iv,
                )

                # Aux partial sums: softmax probs and top-2 one-hot counts.
                nc.vector.tensor_add(stot, stot, sc)
                ft = work.tile([P, MB, E], F32)
                nc.vector.tensor_tensor(
                    out=ft,
                    in0=sc,
                    in1=m8[:, :, 1:2].to_broadcast([P, MB, E]),
                    op=mybir.AluOpType.is_ge,
                )
                nc.vector.tensor_add(ftot, ftot, ft)

            # Reduce the per-(m) accumulators over m -> [P, E].
            sacc_sb = singles.tile([P, E], F32)
            facc_sb = singles.tile([P, E], F32)
            nc.vector.reduce_sum(
                sacc_sb, stot.rearrange("p m e -> p e m"), axis=mybir.AxisListType.X
            )
            nc.vector.reduce_sum(
                facc_sb, ftot.rearrange("p m e -> p e m"), axis=mybir.AxisListType.X
            )
            nc.scalar.dma_start(out=sacc_d[:, :], in_=sacc_sb)
            nc.scalar.dma_start(out=facc_d[:, :], in_=facc_sb)

    nc.finalize()
    return nc


def get_nc():
    if "nc" not in _COMPILED:
        _COMPILED["nc"] = _build()
    return _COMPILED["nc"]


def _host_prep(hidden_states: np.ndarray, weight: np.ndarray):
    x = np.ascontiguousarray(hidden_states, dtype=np.float32).reshape(B * S, H)
    wt = np.ascontiguousarray(weight.T.astype(np.float32, copy=False))
    id8 = np.eye(E, dtype=np.float32)
    xt = np.ascontiguousarray(x.T)  # [H, B*S]
    in_maps = []
    for c in range(N_CORES):
        sl = slice(c * TOK_PER_CORE, (c + 1) * TOK_PER_CORE)
        in_maps.append(
            {"xt": np.ascontiguousarray(xt[:, sl]), "wt": wt, "id8": id8}
        )
    return in_maps


def _host_finish(results):
    topk_w = np.empty((B * S, K), np.float32)
    topk_i = np.empty((B * S, K), np.int32)
    p_sum = np.zeros((B, E), np.float64)
    f_cnt = np.zeros((B, E), np.float64)
    for c, res in enumerate(results):
        # wout/iout layout: [p, j, m, q] for token j*T_B + m*128 + p
        w = res["wout"].reshape(P, NB, MB, K).transpose(1, 2, 0, 3).reshape(-1, K)
        i = res["iout"].reshape(P, NB, MB, K).transpose(1, 2, 0, 3).reshape(-1, K)
        topk_w[c * TOK_PER_CORE:(c + 1) * TOK_PER_CORE] = w
        topk_i[c * TOK_PER_CORE:(c + 1) * TOK_PER_CORE] = i
        b = c // (N_CORES // B)
        p_sum[b] += res["sacc"].astype(np.float64).sum(axis=0)
        f_cnt[b] += res["facc"].astype(np.float64).sum(axis=0)

    p = p_sum / S                      # scores.mean over seq  [B, E]
    f = (E / (K * S)) * f_cnt / S      # scaled one-hot counts, mean over seq
    aux = np.float32(ALPHA * np.mean((f * p).sum(axis=1)))

    return (
        topk_i.reshape(B, S, K),
        topk_w.reshape(B, S, K),
        aux,
    )


def kernel(hidden_states: np.ndarray, weight: np.ndarray, _trace=False,
           _tmpdir=None):
    nc = get_nc()
    in_maps = _host_prep(hidden_states, weight)
    res = run_bass_kernel_spmd(
        nc, in_maps, core_ids=list(range(N_CORES)), trace=_trace,
        tmpdir=_tmpdir,
    )
    out = _host_finish(res.results)
    if _trace:
        return out, res
    return out


# revision 37
# speedup vs baseline: 1.8105x; 1.0261x over previous
"""MoE gate (top-2 routing + seq-aux loss) on 8 Trainium2 NeuronCores.

Strategy (data-parallel over tokens, per sharding hint):
  - Flatten [B=4, S=8192] -> 32768 tokens; each of the 8 cores handles 4096
    tokens (exactly half of one batch row, so per-core aux partial sums
    belong to a single batch).
  - Host pre-transposes each token shard to xT [H=1024, 4096] so the device
    DMAs contiguous rows with H on partitions (PE contraction dim), avoiding
    an fp32 on-chip transpose.  The tiny router weight is host-transposed to
    wT [1024, 8] and replicated.
  - Device per core: fp32 matmul (xT chunk stationary [128h,128t], wT chunk
    moving [128h,8e]) accumulating logits [128t, 8e] in PSUM over 8 H-chunks;
    ScalarE exp; VectorE sum/reciprocal/top-8 sort + index; outputs top-2
    weights (softmax probs) + int32 indices; accumulates per-core partial
    sums of softmax probs and top-2 one-hot counts for the aux loss.
  - Host gathers, reassembles [B, S, 2] outputs, and finishes the scalar
    aux loss from the per-core partial sums.
"""

import os
import sys

import numpy as np

for _p in ("/opt/trn_rl_repo", "/root/.axon_site/_ro/trn_rl_repo"):
    if os.path.isdir(_p) and _p not in sys.path:
        sys.path.insert(0, _p)

import concourse.bacc as bacc
import concourse.mybir as mybir
from concourse.tile import TileContext
from concourse.bass_utils import run_bass_kernel_spmd

# Problem constants (hardcoded per spec).
B, S, H = 4, 8192, 1024
E, K = 8, 2
ALPHA = 0.1
N_CORES = 8
TOK_PER_CORE = (B * S) // N_CORES  # 4096
P = 128
NCH = H // P  # 8 contraction chunks

# Tunables
T_B = 1024                 # tokens per pipeline block
MB = T_B // P              # 128-token subtiles per block
NB = TOK_PER_CORE // T_B   # blocks per core
SB = 512                   # tokens per PSUM sub-block (one matmul stream)
NSB = T_B // SB

F32 = mybir.dt.float32
BF16 = mybir.dt.bfloat16
I32 = mybir.dt.int32
U32 = mybir.dt.uint32

_COMPILED = {}


def _build():
    nc = bacc.Bacc("TRN2", target_bir_lowering=False)

    # Exact fp32 path: weight-stationary matmul streams the token dim
    # (fp32 = 2 internal half-rate passes, but 512-row streams keep the PE
    # warm at 2.4 GHz and hide the tiny weight loads).
    xt_d = nc.dram_tensor("xt", [H, TOK_PER_CORE], F32, kind="ExternalInput")
    wt_d = nc.dram_tensor("wt", [H, E], F32, kind="ExternalInput")
    id8_d = nc.dram_tensor("id8", [E, E], F32, kind="ExternalInput")
    wout_d = nc.dram_tensor("wout", [P, NB * MB * 2], F32, kind="ExternalOutput")
    iout_d = nc.dram_tensor("iout", [P, NB * MB * 2], I32, kind="ExternalOutput")
    sacc_d = nc.dram_tensor("sacc", [P, E], F32, kind="ExternalOutput")
    facc_d = nc.dram_tensor("facc", [P, E], F32, kind="ExternalOutput")

    with TileContext(nc) as tc:
        with (
            tc.tile_pool(name="xin", bufs=3) as xin_pool,
            tc.tile_pool(name="work", bufs=3) as work,
            tc.tile_pool(name="outs", bufs=3) as outs,
            tc.tile_pool(name="singles", bufs=1) as singles,
            tc.tile_pool(name="psum", bufs=2, space="PSUM") as psum_pool,
        ):
            # Router weight, H-chunk-major: wt_sb[p, c, e] = wT[c*128+p, e]
            wt_sb = singles.tile([P, NCH, E], F32)
            nc.sync.dma_start(
                out=wt_sb, in_=wt_d.rearrange("(c p) e -> p c e", p=P)
            )
            id8_sb = singles.tile([E, E], F32)
            nc.sync.dma_start(out=id8_sb, in_=id8_d[:, :])

            # Aux-loss accumulators.
            stot = singles.tile([P, MB, E], F32)
            ftot = singles.tile([P, MB, E], F32)
            nc.vector.memset(stot, 0.0)
            nc.vector.memset(ftot, 0.0)

            for j in range(NB):
                # One DMA per H-chunk so the first matmul can start after
                # ~1/8 of the block has landed.
                xt_sb = xin_pool.tile([P, NCH, T_B], F32)
                for c in range(NCH):
                    nc.sync.dma_start(
                        out=xt_sb[:, c, :],
                        in_=xt_d[c * P:(c + 1) * P, j * T_B:(j + 1) * T_B],
                    )

                # Per-block output staging tiles, filled per sub-block.
                wv = outs.tile([P, MB, 2], F32)
                iv = outs.tile([P, MB, 2], I32)

                MS = SB // P  # 128-token slices per sub-block
                for s in range(NSB):
                    # logitsT [E, SB]: weight-stationary fp32 matmul; tiny
                    # PE transposes flip each 128-token slice token-major.
                    lt_ps = psum_pool.tile([E, SB], F32, tag="ltps")
                    for c in range(NCH):
                        nc.tensor.matmul(
                            lt_ps,
                            lhsT=wt_sb[:, c, :],
                            rhs=xt_sb[:, c, s * SB:(s + 1) * SB],
                            start=(c == 0),
                            stop=(c == NCH - 1),
                        )
                    lt_sb = work.tile([E, SB], F32, tag="ltsb")
                    nc.scalar.copy(lt_sb, lt_ps)
                    lg_ps = psum_pool.tile([P, MS, E], F32, tag="lgps")
                    for m4 in range(MS):
                        nc.tensor.transpose(
                            lg_ps[:, m4, :],
                            lt_sb[:, m4 * P:(m4 + 1) * P],
                            id8_sb,
                        )

                    # softmax with per-token max subtraction (matches the
                    # reference).  negate=True gives -max directly, used as
                    # the per-partition ACT bias (one op per 128-token slice
                    # since bias is a per-partition scalar).
                    nmx = work.tile([P, MS], F32)
                    nc.vector.reduce_max(
                        nmx, lg_ps, axis=mybir.AxisListType.X, negate=True
                    )
                    ex = work.tile([P, MS, E], F32)
                    for m in range(MS):
                        nc.scalar.activation(
                            ex[:, m, :],
                            lg_ps[:, m, :],
                            mybir.ActivationFunctionType.Exp,
                            bias=nmx[:, m:m + 1],
                        )

                    z = work.tile([P, MS], F32)
                    nc.vector.reduce_sum(z, ex, axis=mybir.AxisListType.X)
                    r = work.tile([P, MS], F32)
                    nc.vector.reciprocal(r, z)

                    # full softmax scores (feed the aux accumulator too);
                    # top-k runs on these so rounding ties break like jax's.
                    sc = work.tile([P, MS, E], F32)
                    nc.vector.tensor_mul(sc, ex, r.to_broadcast([P, MS, E]))

                    m8 = work.tile([P, MS, E], F32)
                    i8 = work.tile([P, MS, E], U32)
                    for m in range(MS):
                        nc.vector.max(m8[:, m, :], sc[:, m, :])
                    for m in range(MS):
                        nc.vector.max_index(i8[:, m, :], m8[:, m, :], sc[:, m, :])

                    nc.vector.tensor_copy(wv[:, s * MS:(s + 1) * MS, :],
                                           m8[:, :, 0:2])
                    nc.vector.tensor_copy(iv[:, s * MS:(s + 1) * MS, :],
                                           i8[:, :, 0:2])

                    # Aux partial sums: softmax probs + top-2 one-hot counts.
                    sl = stot[:, s * MS:(s + 1) * MS, :]
                    nc.vector.tensor_add(sl, sl, sc)
                    ft = work.tile([P, MS, E], F32)
                    nc.vector.tensor_tensor(
                        out=ft,
                        in0=sc,
                        in1=m8[:, :, 1:2].to_broadcast([P, MS, E]),
                        op=mybir.AluOpType.is_ge,
                    )
                    fl = ftot[:, s * MS:(s + 1) * MS, :]
                    nc.vector.tensor_add(fl, fl, ft)

                # Outputs leave on the scalar engine's HWDGE ring so they
                # never block the sync ring's input prefetch (a waiting DMA
                # at the ring head stalls everything queued behind it).
                nc.scalar.dma_start(
                    out=wout_d.rearrange("p (j q) -> p j q", j=NB)[:, j, :],
                    in_=wv,
                )
                nc.scalar.dma_start(
                    out=iout_d.rearrange("p (j q) -> p j q", j=NB)[:, j, :],
                    in_=iv,
                )

            # Reduce the per-(m) accumulators over m -> [P, E].
            sacc_sb = singles.tile([P, E], F32)
            facc_sb = singles.tile([P, E], F32)
            nc.vector.reduce_sum(
                sacc_sb, stot.rearrange("p m e -> p e m"), axis=mybir.AxisListType.X
            )
            nc.vector.reduce_sum(
                facc_sb, ftot.rearrange("p m e -> p e m"), axis=mybir.AxisListType.X
            )
            nc.scalar.dma_start(out=sacc_d[:, :], in_=sacc_sb)
            nc.scalar.dma_start(out=facc_d[:, :], in_=facc_sb)

    nc.finalize()
    return nc


def get_nc():
    if "nc" not in _COMPILED:
        _COMPILED["nc"] = _build()
    return _COMPILED["nc"]


def _host_prep(hidden_states: np.ndarray, weight: np.ndarray):
    x = np.ascontiguousarray(hidden_states, dtype=np.float32).reshape(B * S, H)
    wt = np.ascontiguousarray(weight.T.astype(np.float32, copy=False))
    id8 = np.eye(E, dtype=np.float32)
    xt = np.ascontiguousarray(x.T)  # [H, B*S]
    in_maps = []
    for c in range(N_CORES):
        sl = slice(c * TOK_PER_CORE, (c + 1) * TOK_PER_CORE)
        in_maps.append(
            {"xt": np.ascontiguousarray(xt[:, sl]), "wt": wt, "id8": id8}
        )
    return in_maps


def _host_finish(results):
    topk_w = np.empty((B * S, K), np.float32)
    topk_i = np.empty((B * S, K), np.int32)
    p_sum = np.zeros((B, E), np.float64)
    f_cnt = np.zeros((B, E), np.float64)
    for c, res in enumerate(results):
        # wout/iout layout: [p, j, m, q] for token j*T_B + m*128 + p
        w = res["wout"].reshape(P, NB, MB, K).transpose(1, 2, 0, 3).reshape(-1, K)
        i = res["iout"].reshape(P, NB, MB, K).transpose(1, 2, 0, 3).reshape(-1, K)
        topk_w[c * TOK_PER_CORE:(c + 1) * TOK_PER_CORE] = w
        topk_i[c * TOK_PER_CORE:(c + 1) * TOK_PER_CORE] = i
        b = c // (N_CORES // B)
        p_sum[b] += res["sacc"].astype(np.float64).sum(axis=0)
        f_cnt[b] += res["facc"].astype(np.float64).sum(axis=0)

    p = p_sum / S                      # scores.mean over seq  [B, E]
    f = (E / (K * S)) * f_cnt / S      # scaled one-hot counts, mean over seq
    aux = np.float32(ALPHA * np.mean((f * p).sum(axis=1)))

    return (
        topk_i.reshape(B, S, K),
        topk_w.reshape(B, S, K),
        aux,
    )


def kernel(hidden_states: np.ndarray, weight: np.ndarray, _trace=False,
           _tmpdir=None):
    nc = get_nc()
    in_maps = _host_prep(hidden_states, weight)
    res = run_bass_kernel_spmd(
        nc, in_maps, core_ids=list(range(N_CORES)), trace=_trace,
        tmpdir=_tmpdir,
    )
    out = _host_finish(res.results)
    if _trace:
        return out, res
    return out


# revision 42
# speedup vs baseline: 2.1746x; 1.2011x over previous
"""MoE gate (top-2 routing + seq-aux loss) on 8 Trainium2 NeuronCores.

Strategy (data-parallel over tokens, per sharding hint):
  - Flatten [B=4, S=8192] -> 32768 tokens; each of the 8 cores handles 4096
    tokens (exactly half of one batch row, so per-core aux partial sums
    belong to a single batch).
  - Host pre-transposes each token shard to xT [H=1024, 4096] so the device
    DMAs contiguous rows with H on partitions (PE contraction dim), avoiding
    an fp32 on-chip transpose.  The tiny router weight is host-transposed to
    wT [1024, 8] and replicated.
  - Device per core: fp32 matmul (xT chunk stationary [128h,128t], wT chunk
    moving [128h,8e]) accumulating logits [128t, 8e] in PSUM over 8 H-chunks;
    ScalarE exp; VectorE sum/reciprocal/top-8 sort + index; outputs top-2
    weights (softmax probs) + int32 indices; accumulates per-core partial
    sums of softmax probs and top-2 one-hot counts for the aux loss.
  - Host gathers, reassembles [B, S, 2] outputs, and finishes the scalar
    aux loss from the per-core partial sums.
"""

import os
import sys

import numpy as np

for _p in ("/opt/trn_rl_repo", "/root/.axon_site/_ro/trn_rl_repo"):
    if os.path.isdir(_p) and _p not in sys.path:
        sys.path.insert(0, _p)

import concourse.bacc as bacc
import concourse.mybir as mybir
from concourse.tile import TileContext
from concourse.bass_utils import run_bass_kernel_spmd

# Problem constants (hardcoded per spec).
B, S, H = 4, 8192, 1024
E, K = 8, 2
ALPHA = 0.1
N_CORES = 8
TOK_PER_CORE = (B * S) // N_CORES  # 4096
P = 128
NCH = H // P  # 8 contraction chunks

# Tunables
T_B = 2048                 # tokens per pipeline block
MB = T_B // P              # 128-token subtiles per block
NB = TOK_PER_CORE // T_B   # blocks per core
SB = 512                   # tokens per PSUM sub-block (one matmul stream)
NSB = T_B // SB            # sub-blocks = concurrent PE column groups (4)

F32 = mybir.dt.float32
BF16 = mybir.dt.bfloat16
I32 = mybir.dt.int32
U32 = mybir.dt.uint32

_COMPILED = {}


def _build():
    nc = bacc.Bacc("TRN2", target_bir_lowering=False)

    # Exact fp32 path: weight-stationary matmul streams the token dim
    # (fp32 = 2 internal half-rate passes, but 512-row streams keep the PE
    # warm at 2.4 GHz and hide the tiny weight loads).
    xt_d = nc.dram_tensor("xt", [H, TOK_PER_CORE], F32, kind="ExternalInput")
    wt_d = nc.dram_tensor("wt", [H, E], F32, kind="ExternalInput")
    # Identity replicated at partition bases 0/32/64/96 for the per-column-
    # group transposes.
    id8_d = nc.dram_tensor("id8", [P, E], F32, kind="ExternalInput")
    wout_d = nc.dram_tensor("wout", [P, NB * MB * 2], F32, kind="ExternalOutput")
    iout_d = nc.dram_tensor("iout", [P, NB * MB * 2], I32, kind="ExternalOutput")
    sacc_d = nc.dram_tensor("sacc", [P, E], F32, kind="ExternalOutput")
    facc_d = nc.dram_tensor("facc", [P, E], F32, kind="ExternalOutput")

    with TileContext(nc) as tc:
        with (
            tc.tile_pool(name="xin", bufs=3) as xin_pool,
            tc.tile_pool(name="work", bufs=3) as work,
            tc.tile_pool(name="outs", bufs=3) as outs,
            tc.tile_pool(name="singles", bufs=1) as singles,
            tc.tile_pool(name="psum", bufs=2, space="PSUM") as psum_pool,
        ):
            # Router weight, H-chunk-major: wt_sb[p, c, e] = wT[c*128+p, e]
            wt_sb = singles.tile([P, NCH, E], F32)
            nc.sync.dma_start(
                out=wt_sb, in_=wt_d.rearrange("(c p) e -> p c e", p=P)
            )
            id8_sb = singles.tile([P, E], F32)
            nc.sync.dma_start(out=id8_sb, in_=id8_d[:, :])

            # Aux-loss accumulators.
            stot = singles.tile([P, MB, E], F32)
            ftot = singles.tile([P, MB, E], F32)
            nc.vector.memset(stot, 0.0)
            nc.vector.memset(ftot, 0.0)

            for j in range(NB):
                # One DMA per H-chunk so the first matmul can start after
                # ~1/8 of the block has landed.
                xt_sb = xin_pool.tile([P, NCH, T_B], F32)
                for c in range(NCH):
                    nc.sync.dma_start(
                        out=xt_sb[:, c, :],
                        in_=xt_d[c * P:(c + 1) * P, j * T_B:(j + 1) * T_B],
                    )

                # Per-block output staging tiles, filled per half-block.
                wv = outs.tile([P, MB, 2], F32)
                iv = outs.tile([P, MB, 2], I32)

                # Four sub-blocks stream CONCURRENTLY through four PE column
                # groups (tile_position=(0, 32g)), each accumulating its
                # logitsT [8, SB] at partition base 32g of one PSUM bank.
                lt4_ps = psum_pool.tile([P, SB], F32, tag="ltps")
                for c in range(NCH):
                    for g in range(NSB):
                        nc.tensor.matmul(
                            lt4_ps[32 * g:32 * g + E, :],
                            lhsT=wt_sb[:, c, :],
                            rhs=xt_sb[:, c, g * SB:(g + 1) * SB],
                            start=(c == 0),
                            stop=(c == NCH - 1),
                            tile_position=(0, 32 * g),
                            skip_group_check=True,
                        )
                # One full-height copy moves all four logitsT strips to SBUF.
                lt4_sb = work.tile([P, SB], F32, tag="ltsb")
                nc.scalar.copy(lt4_sb, lt4_ps)

                MS = SB // P  # 128-token slices per sub-block
                lg_ps = psum_pool.tile([P, MB, E], F32, tag="lgps")
                for g in range(NSB):
                    for m4 in range(MS):
                        nc.tensor.transpose(
                            lg_ps[:, g * MS + m4, :],
                            lt4_sb[32 * g:32 * g + E, m4 * P:(m4 + 1) * P],
                            id8_sb[32 * g:32 * g + E, :],
                            tile_position=(32 * g, 0),
                        )

                # softmax in half-block chains (keeps the kernel tail short).
                HM = MB // 2
                for h in range(2):
                    lg = lg_ps[:, h * HM:(h + 1) * HM, :]
                    nmx = work.tile([P, HM], F32)
                    nc.vector.reduce_max(
                        nmx, lg, axis=mybir.AxisListType.X, negate=True
                    )
                    ex = work.tile([P, HM, E], F32)
                    for m in range(HM):
                        nc.scalar.activation(
                            ex[:, m, :],
                            lg[:, m, :],
                            mybir.ActivationFunctionType.Exp,
                            bias=nmx[:, m:m + 1],
                        )

                    z = work.tile([P, HM], F32)
                    nc.vector.reduce_sum(z, ex, axis=mybir.AxisListType.X)
                    r = work.tile([P, HM], F32)
                    nc.vector.reciprocal(r, z)

                    # full softmax scores (feed the aux accumulator too);
                    # top-k runs on these so rounding ties break like jax's.
                    sc = work.tile([P, HM, E], F32)
                    nc.vector.tensor_mul(sc, ex, r.to_broadcast([P, HM, E]))

                    m8 = work.tile([P, HM, E], F32)
                    i8 = work.tile([P, HM, E], U32)
                    for m in range(HM):
                        nc.vector.max(m8[:, m, :], sc[:, m, :])
                    for m in range(HM):
                        nc.vector.max_index(i8[:, m, :], m8[:, m, :], sc[:, m, :])

                    nc.vector.tensor_copy(wv[:, h * HM:(h + 1) * HM, :],
                                           m8[:, :, 0:2])
                    nc.vector.tensor_copy(iv[:, h * HM:(h + 1) * HM, :],
                                           i8[:, :, 0:2])

                    # Aux partial sums: softmax probs + top-2 one-hot counts.
                    sl = stot[:, h * HM:(h + 1) * HM, :]
                    nc.vector.tensor_add(sl, sl, sc)
                    ft = work.tile([P, HM, E], F32)
                    nc.vector.tensor_tensor(
                        out=ft,
                        in0=sc,
                        in1=m8[:, :, 1:2].to_broadcast([P, HM, E]),
                        op=mybir.AluOpType.is_ge,
                    )
                    fl = ftot[:, h * HM:(h + 1) * HM, :]
                    nc.vector.tensor_add(fl, fl, ft)

                # Outputs leave on the scalar engine's HWDGE ring so they
                # never block the sync ring's input prefetch (a waiting DMA
                # at the ring head stalls everything queued behind it).
                nc.scalar.dma_start(
                    out=wout_d.rearrange("p (j q) -> p j q", j=NB)[:, j, :],
                    in_=wv,
                )
                nc.scalar.dma_start(
                    out=iout_d.rearrange("p (j q) -> p j q", j=NB)[:, j, :],
                    in_=iv,
                )

            # Reduce the per-(m) accumulators over m -> [P, E].
            sacc_sb = singles.tile([P, E], F32)
            facc_sb = singles.tile([P, E], F32)
            nc.vector.reduce_sum(
                sacc_sb, stot.rearrange("p m e -> p e m"), axis=mybir.AxisListType.X
            )
            nc.vector.reduce_sum(
                facc_sb, ftot.rearrange("p m e -> p e m"), axis=mybir.AxisListType.X
            )
            nc.scalar.dma_start(out=sacc_d[:, :], in_=sacc_sb)
            nc.scalar.dma_start(out=facc_d[:, :], in_=facc_sb)

    nc.finalize()
    return nc


def get_nc():
    if "nc" not in _COMPILED:
        _COMPILED["nc"] = _build()
    return _COMPILED["nc"]


def _host_prep(hidden_states: np.ndarray, weight: np.ndarray):
    x = np.ascontiguousarray(hidden_states, dtype=np.float32).reshape(B * S, H)
    wt = np.ascontiguousarray(weight.T.astype(np.float32, copy=False))
    id8 = np.zeros((P, E), np.float32)
    for g in range(NSB):
        id8[32 * g:32 * g + E] = np.eye(E, dtype=np.float32)
    xt = np.ascontiguousarray(x.T)  # [H, B*S]
    in_maps = []
    for c in range(N_CORES):
        sl = slice(c * TOK_PER_CORE, (c + 1) * TOK_PER_CORE)
        in_maps.append(
            {"xt": np.ascontiguousarray(xt[:, sl]), "wt": wt, "id8": id8}
        )
    return in_maps


def _host_finish(results):
    topk_w = np.empty((B * S, K), np.float32)
    topk_i = np.empty((B * S, K), np.int32)
    p_sum = np.zeros((B, E), np.float64)
    f_cnt = np.zeros((B, E), np.float64)
    for c, res in enumerate(results):
        # wout/iout layout: [p, j, m, q] for token j*T_B + m*128 + p
        w = res["wout"].reshape(P, NB, MB, K).transpose(1, 2, 0, 3).reshape(-1, K)
        i = res["iout"].reshape(P, NB, MB, K).transpose(1, 2, 0, 3).reshape(-1, K)
        topk_w[c * TOK_PER_CORE:(c + 1) * TOK_PER_CORE] = w
        topk_i[c * TOK_PER_CORE:(c + 1) * TOK_PER_CORE] = i
        b = c // (N_CORES // B)
        p_sum[b] += res["sacc"].astype(np.float64).sum(axis=0)
        f_cnt[b] += res["facc"].astype(np.float64).sum(axis=0)

    p = p_sum / S                      # scores.mean over seq  [B, E]
    f = (E / (K * S)) * f_cnt / S      # scaled one-hot counts, mean over seq
    aux = np.float32(ALPHA * np.mean((f * p).sum(axis=1)))

    return (
        topk_i.reshape(B, S, K),
        topk_w.reshape(B, S, K),
        aux,
    )


def kernel(hidden_states: np.ndarray, weight: np.ndarray, _trace=False,
           _tmpdir=None):
    nc = get_nc()
    in_maps = _host_prep(hidden_states, weight)
    res = run_bass_kernel_spmd(
        nc, in_maps, core_ids=list(range(N_CORES)), trace=_trace,
        tmpdir=_tmpdir,
    )
    out = _host_finish(res.results)
    if _trace:
        return out, res
    return out
